# revision 23
# baseline (speedup 1.0000x reference)
"""Trainium2 Bass kernel for nn_DetectorKe_652835029279 (Gaussian-mixture
log-likelihood detector: weighted logsumexp over 256 Mahalanobis distances).

Math: ll_i = log sum_j coef_j * exp(-0.5 * (x_i-c_j)^T A_j (x_i-c_j)) - thr
    = logsumexp_j( -0.5 * x^T A_j x + x . (A_j c_j) + bias_j )

Split A = diag(A) + offdiag(A). Since cov_inv_sqrt = I + 0.02 G, the
off-diagonal entries of A are small (~0.03) and their pair-product terms
tolerate fp8: the 512 off-diagonal pair slots (cyclic shifts 1..16) run as
fp8e4m3 DoubleRow matmuls (2 K-rows per partition per cycle, 0.5 cyc/row on
the PE), while the diagonal x^2 terms, the x-linear terms and the bias run
in one float32r chunk. Measured end-to-end error of the fp8 path: ~2.5e-3
relative (gate is 2e-2).

Per 512-row tile:
  - DMA the host-prepped X^T stack X4T [x; x; x; rot16(x)] (no on-chip
    transposes) + the x rows of the precise chunk.
  - 4 K=128 f32r selection matmuls build rotated X^T copies in PSUM
    (K=128 stationaries everywhere - mixing PE tile sizes serializes the
    weight-load pipeline, measured +200ns per switch).
  - DVE multiplies xt4 by each rotation straight out of PSUM, writing
    fp8e4 products in DoubleRow [p, ktile, row] layout.
  - GpSimd (idle Pool queue) computes the diagonal x^2 products.
  - Main matmuls per 128-row group: 1 f32r (diag+linear+bias, K=128) +
    2 fp8 DoubleRow (512 off-diag slots as 2x K=256) accumulating in PSUM;
    f32r and fp8 matmuls are batched to minimize PE mode switches.
  - ACT exp with fused free-dim accumulate -> per-row sums; Ln + transpose
    + contiguous DMA out at the end.
The loop is software-pipelined 3 stages deep (DMA/GpSimd -> SEL/DVE ->
main/exp) so the PE never waits at steady state.
"""
import sys

if "/opt/trn_rl_repo" not in sys.path:
    sys.path.insert(0, "/opt/trn_rl_repo")

import numpy as np

N, D, M = 131072, 32, 256
NCORES = 8
NC_ROWS = N // NCORES          # 16384
TILE_ROWS = 512
NTILES = NC_ROWS // TILE_ROWS  # 32
NGROUPS = NC_ROWS // 128       # 128

_PROGRAM = None


def _build_program():
    import concourse.bacc as bacc
    import concourse.mybir as mybir
    import concourse.tile as tile

    f32 = mybir.dt.float32
    f32r = mybir.dt.float32r
    f8 = mybir.dt.float8e4
    AF = mybir.ActivationFunctionType
    DR = mybir.MatmulPerfMode.DoubleRow

    nc = bacc.Bacc(None, target_bir_lowering=False)
    X4_d = nc.dram_tensor("X4", [128, NC_ROWS], f32r, kind="ExternalInput")
    UP_d = nc.dram_tensor("UP", [128, M], f32r, kind="ExternalInput")
    U8_d = nc.dram_tensor("U8", [128, 2, 2, M], f8, kind="ExternalInput")
    SEL_d = nc.dram_tensor("SEL", [128, 512], f32r, kind="ExternalInput")
    PAD_d = nc.dram_tensor("PAD", [64, TILE_ROWS], f32r, kind="ExternalInput")
    EYE_d = nc.dram_tensor("EYE", [128, 128], f32, kind="ExternalInput")
    OUT_d = nc.dram_tensor("out", [NC_ROWS], f32, kind="ExternalOutput")

    with tile.TileContext(nc) as tc:
        with (
            tc.tile_pool(name="const", bufs=1) as constp,
            tc.tile_pool(name="xt4", bufs=4) as xt4pool,
            tc.tile_pool(name="xxp", bufs=2) as xxpool,
            tc.tile_pool(name="expp", bufs=4) as exppool,
            tc.tile_pool(name="sumsp", bufs=1) as sumspool,
            tc.tile_pool(name="finp", bufs=1) as finpool,
            tc.tile_pool(name="ps_rot", bufs=3, space="PSUM") as ps_rot,
            tc.tile_pool(name="ps_main", bufs=5, space="PSUM") as ps_main,
        ):
            # SEL is needed first (by the first selection matmul) - give it
            # its own queue (gpsimd) so it lands while xt4(0) streams on the
            # sync queue; the other constants ride the scalar engine's queue.
            SEL_sb = constp.tile([128, 512], f32r)
            nc.gpsimd.dma_start(SEL_sb[:], SEL_d[:])
            UP_sb = constp.tile([128, M], f32r)
            nc.scalar.dma_start(UP_sb[:], UP_d[:])
            U8_sb = constp.tile([128, 2, 2, M], f8)
            nc.scalar.dma_start(U8_sb[:], U8_d[:])
            EYE_sb = constp.tile([128, 128], f32)
            nc.scalar.dma_start(EYE_sb[:], EYE_d[:])

            sums_sb = sumspool.tile([128, NGROUPS], f32)

            # persistent precise-chunk tiles (3 rotating buffers):
            #   rows 0:32  = x^2     (GpSimd, per tile)
            #   rows 32:64 = x       (DMA'd per tile)
            #   row 64     = ones    (PAD, written once)
            #   rows 65:128= zeros   (PAD, written once)
            NC4 = 3
            c4_tiles = []
            for i in range(NC4):
                c4 = xt4pool.tile(
                    [128, TILE_ROWS], f32r, tag=f"c4P{i}", bufs=1, name=f"c4_p{i}"
                )
                nc.scalar.dma_start(c4[64:128, :], PAD_d[:])
                c4_tiles.append(c4)

            # 3-stage software pipeline:
            #   A(t):   input DMAs + GpSimd x^2 products for tile t
            #   B(t-1): selection matmuls (PE) + fp8 pair products (DVE)
            #   C(t-2): main accumulating matmuls (PE) + exp (ACT)
            stageA = {}
            stageB = {}
            for t in range(NTILES + 2):
                if t < NTILES:
                    cols = slice(t * TILE_ROWS, (t + 1) * TILE_ROWS)
                    xt4_t = xt4pool.tile([128, TILE_ROWS], f32r, tag="xt4")
                    nc.sync.dma_start(xt4_t[:], X4_d[:, cols])
                    c4_t = c4_tiles[t % NC4]
                    nc.sync.dma_start(c4_t[32:64, :], X4_d[0:32, cols])
                    # diagonal x^2 products on the idle GpSimd/Pool queue
                    nc.gpsimd.tensor_mul(
                        c4_t[0:32, :], xt4_t[0:32, :], xt4_t[0:32, :]
                    )
                    stageA[t] = (xt4_t, c4_t)

                tb = t - 1
                if 0 <= tb < NTILES:
                    xt4_b, c4_b = stageA.pop(tb)
                    # rotated copies via K=128 selection matmuls; DVE builds
                    # fp8 DoubleRow product bundles [128, 2, rows]
                    prod8 = []
                    for c in range(2):
                        p8 = xxpool.tile([128, 2, TILE_ROWS], f8, tag=f"p8{c}")
                        prod8.append(p8)
                    for g in range(4):
                        rotps = ps_rot.tile([128, TILE_ROWS], f32, tag="rot")
                        nc.tensor.matmul(
                            rotps[:],
                            SEL_sb[:, 128 * g : 128 * (g + 1)],
                            xt4_b[:],
                            start=True,
                            stop=True,
                        )
                        nc.vector.tensor_mul(
                            prod8[g // 2][:, g % 2, :], xt4_b[:], rotps[:]
                        )
                    stageB[tb] = (prod8, c4_b)

                tcm = t - 2
                if tcm >= 0:
                    prod8, c4_b = stageB.pop(tcm)
                    # main matmuls for tile t-2: all f32r first, then all fp8
                    # (PE dtype-mode switches serialize the pipeline)
                    pstiles = []
                    for half in range(2):
                        psmain = ps_main.tile([128, 2 * M], f32, tag="main")
                        pstiles.append(psmain)
                        for s2 in range(2):
                            sub = half * 2 + s2
                            # start=True arms zero-on-first-touch for the WHOLE
                            # 2KB bank; arm it once (s2=0) - s2=1's first write
                            # consumes the pending-zero of its own bytes.
                            # K=65: rows 65:128 of the precise chunk are all
                            # zero coefficients - a shorter stationary load
                            nc.tensor.matmul(
                                psmain[:, s2 * M : (s2 + 1) * M],
                                c4_b[0:65, sub * 128 : (sub + 1) * 128],
                                UP_sb[0:65, :],
                                start=(s2 == 0),
                                stop=False,
                                skip_group_check=True,
                            )
                    for half in range(2):
                        psmain = pstiles[half]
                        for s2 in range(2):
                            sub = half * 2 + s2
                            for c in range(2):
                                nc.tensor.matmul(
                                    psmain[:, s2 * M : (s2 + 1) * M],
                                    prod8[c][:, :, sub * 128 : (sub + 1) * 128],
                                    U8_sb[:, c],
                                    perf_mode=DR,
                                    start=False,
                                    stop=(c == 1),
                                )
                    for half in range(2):
                        psmain = pstiles[half]
                        for s2 in range(2):
                            sub = half * 2 + s2
                            expsc = exppool.tile([128, M], f32, tag="exp")
                            col = tcm * 4 + sub
                            nc.scalar.activation(
                                expsc[:],
                                psmain[:, s2 * M : (s2 + 1) * M],
                                AF.Exp,
                                accum_out=sums_sb[:, col : col + 1],
                            )

            # epilogue: ll^T = Ln(sums); transpose; contiguous DMA out
            llT = finpool.tile([128, NGROUPS], f32)
            nc.scalar.activation(llT[:], sums_sb[:], AF.Ln)
            llps = ps_rot.tile([128, 128], f32, tag="rot")
            nc.tensor.transpose(llps[:], llT[:], EYE_sb[:])
            ll_sb = finpool.tile([128, 128], f32)
            nc.scalar.copy(ll_sb[:], llps[:])
            nc.sync.dma_start(OUT_d.rearrange("(c p) -> c p", c=128), ll_sb[:])

    nc.compile()
    return nc


def _host_prep(center, cov_inv_sqrt, weight, threshold):
    import ml_dtypes

    L = np.asarray(cov_inv_sqrt, dtype=np.float64)
    w = np.abs(np.asarray(weight, dtype=np.float64))
    pr = w / w.sum()
    A = np.einsum("mij,mkj->mik", L, L)
    sign, logdet = np.linalg.slogdet(A)
    logcoef = np.log(pr) + 0.5 * logdet
    c64 = np.asarray(center, dtype=np.float64)
    Ac = np.einsum("mkl,ml->mk", A, c64)
    term3 = np.einsum("mk,mk->m", c64, Ac)
    bias = logcoef - 0.5 * term3 - float(np.asarray(threshold).reshape(-1)[0])

    d = np.arange(32)
    f8 = ml_dtypes.float8_e4m3

    # precise chunk: diag x^2 rows, x-linear rows, bias row
    UP = np.zeros((128, M), np.float32)
    UP[0:32, :] = (-0.5 * A[:, d, d].T).astype(np.float32)
    UP[32:64, :] = Ac.T.astype(np.float32)
    UP[64, :] = bias.astype(np.float32)

    # fp8 bundles: shift groups g = 2c + k cover shifts 4g+1 .. 4g+4;
    # partition blocks 0..2 use plain-x left factors (shifts 4g+1..4g+3),
    # block 3 uses the rot16 left factor (shift 4g+4).
    U8 = np.zeros((128, 2, 2, M), np.float32)
    SEL = np.zeros((128, 512), np.float32)
    for g in range(4):
        c, k = divmod(g, 2)
        for blk in range(4):
            if blk < 3:
                s = 4 * g + blk + 1
                a = d
                b = (d + s) % 32
            else:
                s = 4 * g + 4
                a = (d + 16) % 32
                b = (a + s) % 32
            mult = 1.0 if s == 16 else 2.0
            U8[32 * blk + d, c, k, :] = (-0.5 * mult * A[:, a, b].T).astype(
                np.float32
            )
            SEL[b, 128 * g + 32 * blk + d] = 1.0
    U8 = U8.astype(f8)

    PAD = np.zeros((64, TILE_ROWS), np.float32)
    PAD[0, :] = 1.0
    EYE = np.eye(128, dtype=np.float32)
    return UP, U8, SEL, PAD, EYE


def _host_x4t(X):
    """[128, N]: rows 0:96 = three copies of X^T, rows 96:128 = rot16(X^T)."""
    X4T = np.empty((128, X.shape[0]), np.float32)
    XT = X.T
    X4T[0:32] = XT
    X4T[32:64] = XT
    X4T[64:96] = XT
    X4T[96:128] = XT[(np.arange(32) + 16) % 32]
    return X4T


def kernel(X, center, cov_inv_sqrt, weight, threshold):
    global _PROGRAM
    from concourse.bass_utils import run_bass_kernel_spmd

    X = np.ascontiguousarray(np.asarray(X, dtype=np.float32))
    UP, U8, SEL, PAD, EYE = _host_prep(center, cov_inv_sqrt, weight, threshold)
    X4T = _host_x4t(X)

    if _PROGRAM is None:
        _PROGRAM = _build_program()
    nc = _PROGRAM

    in_maps = []
    for k in range(NCORES):
        in_maps.append(
            {
                "X4": np.ascontiguousarray(
                    X4T[:, k * NC_ROWS : (k + 1) * NC_ROWS]
                ),
                "UP": UP,
                "U8": U8,
                "SEL": SEL,
                "PAD": PAD,
                "EYE": EYE,
            }
        )
    res = run_bass_kernel_spmd(nc, in_maps, list(range(NCORES)))
    out = np.concatenate([res.results[k]["out"] for k in range(NCORES)])
    return out.astype(np.float32)


# revision 24
# speedup vs baseline: 1.0550x; 1.0550x over previous
"""Trainium2 Bass kernel for nn_DetectorKe_652835029279 (Gaussian-mixture
log-likelihood detector: weighted logsumexp over 256 Mahalanobis distances).

Math: ll_i = log sum_j coef_j * exp(-0.5 * (x_i-c_j)^T A_j (x_i-c_j)) - thr
    = logsumexp_j( -0.5 * x^T A_j x + x . (A_j c_j) + bias_j )

Split A = diag(A) + offdiag(A). Since cov_inv_sqrt = I + 0.02 G, the
off-diagonal entries of A are small (~0.03) and their pair-product terms
tolerate fp8: the 512 off-diagonal pair slots (cyclic shifts 1..16) run as
fp8e4m3 DoubleRow matmuls (2 K-rows per partition per cycle, 0.5 cyc/row on
the PE), while the diagonal x^2 terms, the x-linear terms and the bias run
in one float32r chunk. Measured end-to-end error of the fp8 path: ~2.5e-3
relative (gate is 2e-2).

Per 512-row tile:
  - DMA the host-prepped X^T stack X4T [x; x; x; rot16(x)] (no on-chip
    transposes) + the x rows of the precise chunk.
  - 4 K=128 f32r selection matmuls build rotated X^T copies in PSUM
    (K=128 stationaries everywhere - mixing PE tile sizes serializes the
    weight-load pipeline, measured +200ns per switch).
  - DVE multiplies xt4 by each rotation straight out of PSUM, writing
    fp8e4 products in DoubleRow [p, ktile, row] layout.
  - GpSimd (idle Pool queue) computes the diagonal x^2 products.
  - Main matmuls per 128-row group: 1 f32r (diag+linear+bias, K=128) +
    2 fp8 DoubleRow (512 off-diag slots as 2x K=256) accumulating in PSUM;
    f32r and fp8 matmuls are batched to minimize PE mode switches.
  - ACT exp with fused free-dim accumulate -> per-row sums; Ln + transpose
    + contiguous DMA out at the end.
The loop is software-pipelined 3 stages deep (DMA/GpSimd -> SEL/DVE ->
main/exp) so the PE never waits at steady state.
"""
import sys

if "/opt/trn_rl_repo" not in sys.path:
    sys.path.insert(0, "/opt/trn_rl_repo")

import numpy as np

N, D, M = 131072, 32, 256
NCORES = 8
NC_ROWS = N // NCORES          # 16384
TILE_ROWS = 512
NTILES = NC_ROWS // TILE_ROWS  # 32
NGROUPS = NC_ROWS // 128       # 128

_PROGRAM = None


def _build_program():
    import concourse.bacc as bacc
    import concourse.mybir as mybir
    import concourse.tile as tile

    f32 = mybir.dt.float32
    f32r = mybir.dt.float32r
    f8 = mybir.dt.float8e4
    AF = mybir.ActivationFunctionType
    DR = mybir.MatmulPerfMode.DoubleRow

    nc = bacc.Bacc(None, target_bir_lowering=False)
    X4_d = nc.dram_tensor("X4", [128, NC_ROWS], f32r, kind="ExternalInput")
    UP_d = nc.dram_tensor("UP", [128, M], f32r, kind="ExternalInput")
    U8_d = nc.dram_tensor("U8", [128, 2, 2, M], f8, kind="ExternalInput")
    SEL_d = nc.dram_tensor("SEL", [128, 512], f32r, kind="ExternalInput")
    PAD_d = nc.dram_tensor("PAD", [64, TILE_ROWS], f32r, kind="ExternalInput")
    EYE_d = nc.dram_tensor("EYE", [128, 128], f32, kind="ExternalInput")
    OUT_d = nc.dram_tensor("out", [NC_ROWS], f32, kind="ExternalOutput")

    with tile.TileContext(nc) as tc:
        with (
            tc.tile_pool(name="const", bufs=1) as constp,
            tc.tile_pool(name="xt4", bufs=4) as xt4pool,
            tc.tile_pool(name="xxp", bufs=2) as xxpool,
            tc.tile_pool(name="expp", bufs=4) as exppool,
            tc.tile_pool(name="sumsp", bufs=1) as sumspool,
            tc.tile_pool(name="finp", bufs=1) as finpool,
            tc.tile_pool(name="ps_rot", bufs=4, space="PSUM") as ps_rot,
            tc.tile_pool(name="ps_main", bufs=4, space="PSUM") as ps_main,
        ):
            # SEL is needed first (by the first selection matmul) - give it
            # its own queue (gpsimd) so it lands while xt4(0) streams on the
            # sync queue; the other constants ride the scalar engine's queue.
            SEL_sb = constp.tile([128, 512], f32r)
            nc.gpsimd.dma_start(SEL_sb[:], SEL_d[:])
            UP_sb = constp.tile([128, M], f32r)
            nc.scalar.dma_start(UP_sb[:], UP_d[:])
            U8_sb = constp.tile([128, 2, 2, M], f8)
            nc.scalar.dma_start(U8_sb[:], U8_d[:])
            EYE_sb = constp.tile([128, 128], f32)
            nc.scalar.dma_start(EYE_sb[:], EYE_d[:])

            sums_sb = sumspool.tile([128, NGROUPS], f32)

            # persistent precise-chunk tiles (3 rotating buffers):
            #   rows 0:32  = x^2     (GpSimd, per tile)
            #   rows 32:64 = x       (DMA'd per tile)
            #   row 64     = ones    (PAD, written once)
            #   rows 65:128= zeros   (PAD, written once)
            NC4 = 3
            c4_tiles = []
            for i in range(NC4):
                c4 = xt4pool.tile(
                    [128, TILE_ROWS], f32r, tag=f"c4P{i}", bufs=1, name=f"c4_p{i}"
                )
                nc.scalar.dma_start(c4[64:128, :], PAD_d[:])
                c4_tiles.append(c4)

            # 3-stage software pipeline:
            #   A(t):   input DMAs + GpSimd x^2 products for tile t
            #   B(t-1): selection matmuls (PE) + fp8 pair products (DVE)
            #   C(t-2): main accumulating matmuls (PE) + exp (ACT)
            stageA = {}
            stageB = {}
            for t in range(NTILES + 2):
                if t < NTILES:
                    cols = slice(t * TILE_ROWS, (t + 1) * TILE_ROWS)
                    xt4_t = xt4pool.tile([128, TILE_ROWS], f32r, tag="xt4")
                    nc.sync.dma_start(xt4_t[:], X4_d[:, cols])
                    c4_t = c4_tiles[t % NC4]
                    nc.sync.dma_start(c4_t[32:64, :], X4_d[0:32, cols])
                    # diagonal x^2 products on the idle GpSimd/Pool queue
                    nc.gpsimd.tensor_mul(
                        c4_t[0:32, :], xt4_t[0:32, :], xt4_t[0:32, :]
                    )
                    stageA[t] = (xt4_t, c4_t)

                tb = t - 1
                if 0 <= tb < NTILES:
                    xt4_b, c4_b = stageA.pop(tb)
                    # rotated copies via K=128 selection matmuls; DVE builds
                    # fp8 DoubleRow product bundles [128, 2, rows]
                    prod8 = []
                    for c in range(2):
                        p8 = xxpool.tile([128, 2, TILE_ROWS], f8, tag=f"p8{c}")
                        prod8.append(p8)
                    for g in range(4):
                        rotps = ps_rot.tile([128, TILE_ROWS], f32, tag="rot")
                        nc.tensor.matmul(
                            rotps[:],
                            SEL_sb[:, 128 * g : 128 * (g + 1)],
                            xt4_b[:],
                            start=True,
                            stop=True,
                        )
                        nc.vector.tensor_mul(
                            prod8[g // 2][:, g % 2, :], xt4_b[:], rotps[:]
                        )
                    stageB[tb] = (prod8, c4_b)

                tcm = t - 2
                if tcm >= 0:
                    prod8, c4_b = stageB.pop(tcm)
                    # main matmuls for tile t-2: all f32r first, then all fp8
                    # (PE dtype-mode switches serialize the pipeline)
                    pstiles = []
                    for half in range(2):
                        psmain = ps_main.tile([128, 2 * M], f32, tag="main")
                        pstiles.append(psmain)
                        for s2 in range(2):
                            sub = half * 2 + s2
                            # start=True arms zero-on-first-touch for the WHOLE
                            # 2KB bank; arm it once (s2=0) - s2=1's first write
                            # consumes the pending-zero of its own bytes.
                            # K=65: rows 65:128 of the precise chunk are all
                            # zero coefficients - a shorter stationary load
                            nc.tensor.matmul(
                                psmain[:, s2 * M : (s2 + 1) * M],
                                c4_b[0:65, sub * 128 : (sub + 1) * 128],
                                UP_sb[0:65, :],
                                start=(s2 == 0),
                                stop=False,
                                skip_group_check=True,
                            )
                    for half in range(2):
                        psmain = pstiles[half]
                        for s2 in range(2):
                            sub = half * 2 + s2
                            for c in range(2):
                                nc.tensor.matmul(
                                    psmain[:, s2 * M : (s2 + 1) * M],
                                    prod8[c][:, :, sub * 128 : (sub + 1) * 128],
                                    U8_sb[:, c],
                                    perf_mode=DR,
                                    start=False,
                                    stop=(c == 1),
                                )
                    for half in range(2):
                        psmain = pstiles[half]
                        for s2 in range(2):
                            sub = half * 2 + s2
                            expsc = exppool.tile([128, M], f32, tag="exp")
                            col = tcm * 4 + sub
                            nc.scalar.activation(
                                expsc[:],
                                psmain[:, s2 * M : (s2 + 1) * M],
                                AF.Exp,
                                accum_out=sums_sb[:, col : col + 1],
                            )

            # epilogue: ll^T = Ln(sums); transpose; contiguous DMA out
            llT = finpool.tile([128, NGROUPS], f32)
            nc.scalar.activation(llT[:], sums_sb[:], AF.Ln)
            llps = ps_rot.tile([128, 128], f32, tag="rot")
            nc.tensor.transpose(llps[:], llT[:], EYE_sb[:])
            ll_sb = finpool.tile([128, 128], f32)
            nc.scalar.copy(ll_sb[:], llps[:])
            nc.sync.dma_start(OUT_d.rearrange("(c p) -> c p", c=128), ll_sb[:])

    nc.compile()
    return nc


def _host_prep(center, cov_inv_sqrt, weight, threshold):
    import ml_dtypes

    L = np.asarray(cov_inv_sqrt, dtype=np.float64)
    w = np.abs(np.asarray(weight, dtype=np.float64))
    pr = w / w.sum()
    A = np.einsum("mij,mkj->mik", L, L)
    sign, logdet = np.linalg.slogdet(A)
    logcoef = np.log(pr) + 0.5 * logdet
    c64 = np.asarray(center, dtype=np.float64)
    Ac = np.einsum("mkl,ml->mk", A, c64)
    term3 = np.einsum("mk,mk->m", c64, Ac)
    bias = logcoef - 0.5 * term3 - float(np.asarray(threshold).reshape(-1)[0])

    d = np.arange(32)
    f8 = ml_dtypes.float8_e4m3

    # precise chunk: diag x^2 rows, x-linear rows, bias row
    UP = np.zeros((128, M), np.float32)
    UP[0:32, :] = (-0.5 * A[:, d, d].T).astype(np.float32)
    UP[32:64, :] = Ac.T.astype(np.float32)
    UP[64, :] = bias.astype(np.float32)

    # fp8 bundles: shift groups g = 2c + k cover shifts 4g+1 .. 4g+4;
    # partition blocks 0..2 use plain-x left factors (shifts 4g+1..4g+3),
    # block 3 uses the rot16 left factor (shift 4g+4).
    U8 = np.zeros((128, 2, 2, M), np.float32)
    SEL = np.zeros((128, 512), np.float32)
    for g in range(4):
        c, k = divmod(g, 2)
        for blk in range(4):
            if blk < 3:
                s = 4 * g + blk + 1
                a = d
                b = (d + s) % 32
            else:
                s = 4 * g + 4
                a = (d + 16) % 32
                b = (a + s) % 32
            mult = 1.0 if s == 16 else 2.0
            U8[32 * blk + d, c, k, :] = (-0.5 * mult * A[:, a, b].T).astype(
                np.float32
            )
            SEL[b, 128 * g + 32 * blk + d] = 1.0
    U8 = U8.astype(f8)

    PAD = np.zeros((64, TILE_ROWS), np.float32)
    PAD[0, :] = 1.0
    EYE = np.eye(128, dtype=np.float32)
    return UP, U8, SEL, PAD, EYE


def _host_x4t(X):
    """[128, N]: rows 0:96 = three copies of X^T, rows 96:128 = rot16(X^T)."""
    X4T = np.empty((128, X.shape[0]), np.float32)
    XT = X.T
    X4T[0:32] = XT
    X4T[32:64] = XT
    X4T[64:96] = XT
    X4T[96:128] = XT[(np.arange(32) + 16) % 32]
    return X4T


def kernel(X, center, cov_inv_sqrt, weight, threshold):
    global _PROGRAM
    from concourse.bass_utils import run_bass_kernel_spmd

    X = np.ascontiguousarray(np.asarray(X, dtype=np.float32))
    UP, U8, SEL, PAD, EYE = _host_prep(center, cov_inv_sqrt, weight, threshold)
    X4T = _host_x4t(X)

    if _PROGRAM is None:
        _PROGRAM = _build_program()
    nc = _PROGRAM

    in_maps = []
    for k in range(NCORES):
        in_maps.append(
            {
                "X4": np.ascontiguousarray(
                    X4T[:, k * NC_ROWS : (k + 1) * NC_ROWS]
                ),
                "UP": UP,
                "U8": U8,
                "SEL": SEL,
                "PAD": PAD,
                "EYE": EYE,
            }
        )
    res = run_bass_kernel_spmd(nc, in_maps, list(range(NCORES)))
    out = np.concatenate([res.results[k]["out"] for k in range(NCORES)])
    return out.astype(np.float32)


# revision 26
# speedup vs baseline: 1.0751x; 1.0191x over previous
"""Trainium2 Bass kernel for nn_DetectorKe_652835029279 (Gaussian-mixture
log-likelihood detector: weighted logsumexp over 256 Mahalanobis distances).

Math: ll_i = log sum_j coef_j * exp(-0.5 * (x_i-c_j)^T A_j (x_i-c_j)) - thr
    = logsumexp_j( -0.5 * x^T A_j x + x . (A_j c_j) + bias_j )

Split A = diag(A) + offdiag(A). Since cov_inv_sqrt = I + 0.02 G, the
off-diagonal entries of A are small (~0.03) and their pair-product terms
tolerate fp8: the 512 off-diagonal pair slots (cyclic shifts 1..16) run as
fp8e4m3 DoubleRow matmuls (2 K-rows per partition per cycle, 0.5 cyc/row on
the PE), while the diagonal x^2 terms, the x-linear terms and the bias run
in one float32r chunk. Measured end-to-end error of the fp8 path: ~2.5e-3
relative (gate is 2e-2).

Per 512-row tile:
  - DMA the host-prepped X^T stack X4T [x; x; x; rot16(x)] (no on-chip
    transposes) + the x rows of the precise chunk.
  - 4 K=128 f32r selection matmuls build rotated X^T copies in PSUM
    (K=128 stationaries everywhere - mixing PE tile sizes serializes the
    weight-load pipeline, measured +200ns per switch).
  - DVE multiplies xt4 by each rotation straight out of PSUM, writing
    fp8e4 products in DoubleRow [p, ktile, row] layout.
  - GpSimd (idle Pool queue) computes the diagonal x^2 products.
  - Main matmuls per 128-row group: 1 f32r (diag+linear+bias, K=128) +
    2 fp8 DoubleRow (512 off-diag slots as 2x K=256) accumulating in PSUM;
    f32r and fp8 matmuls are batched to minimize PE mode switches.
  - ACT exp with fused free-dim accumulate -> per-row sums; Ln + transpose
    + contiguous DMA out at the end.
The loop is software-pipelined 3 stages deep (DMA/GpSimd -> SEL/DVE ->
main/exp) so the PE never waits at steady state.
"""
import sys

if "/opt/trn_rl_repo" not in sys.path:
    sys.path.insert(0, "/opt/trn_rl_repo")

import numpy as np

N, D, M = 131072, 32, 256
NCORES = 8
NC_ROWS = N // NCORES          # 16384
TILE_ROWS = 512
NTILES = NC_ROWS // TILE_ROWS  # 32
NGROUPS = NC_ROWS // 128       # 128

_PROGRAM = None


def _build_program():
    import concourse.bacc as bacc
    import concourse.mybir as mybir
    import concourse.tile as tile

    f32 = mybir.dt.float32
    f32r = mybir.dt.float32r
    f8 = mybir.dt.float8e4
    AF = mybir.ActivationFunctionType
    DR = mybir.MatmulPerfMode.DoubleRow

    nc = bacc.Bacc(None, target_bir_lowering=False)
    X4_d = nc.dram_tensor("X4", [128, NC_ROWS], f32r, kind="ExternalInput")
    UP_d = nc.dram_tensor("UP", [128, M], f32r, kind="ExternalInput")
    U8_d = nc.dram_tensor("U8", [128, 2, 2, M], f8, kind="ExternalInput")
    SEL_d = nc.dram_tensor("SEL", [128, 512], f32r, kind="ExternalInput")
    PAD_d = nc.dram_tensor("PAD", [64, TILE_ROWS], f32r, kind="ExternalInput")
    EYE_d = nc.dram_tensor("EYE", [128, 128], f32, kind="ExternalInput")
    OUT_d = nc.dram_tensor("out", [NC_ROWS], f32, kind="ExternalOutput")

    with tile.TileContext(nc) as tc:
        with (
            tc.tile_pool(name="const", bufs=1) as constp,
            tc.tile_pool(name="xt4", bufs=4) as xt4pool,
            tc.tile_pool(name="xxp", bufs=2) as xxpool,
            tc.tile_pool(name="expp", bufs=4) as exppool,
            tc.tile_pool(name="sumsp", bufs=1) as sumspool,
            tc.tile_pool(name="finp", bufs=1) as finpool,
            tc.tile_pool(name="ps_rot", bufs=2, space="PSUM") as ps_rot,
            tc.tile_pool(name="ps_main", bufs=4, space="PSUM") as ps_main,
        ):
            # SEL is needed first (by the first selection matmul) - give it
            # its own queue (gpsimd) so it lands while xt4(0) streams on the
            # sync queue; the other constants ride the scalar engine's queue.
            SEL_sb = constp.tile([128, 512], f32r)
            nc.gpsimd.dma_start(SEL_sb[:], SEL_d[:])
            UP_sb = constp.tile([128, M], f32r)
            nc.scalar.dma_start(UP_sb[:], UP_d[:])
            U8_sb = constp.tile([128, 2, 2, M], f8)
            nc.scalar.dma_start(U8_sb[:], U8_d[:])
            EYE_sb = constp.tile([128, 128], f32)
            nc.scalar.dma_start(EYE_sb[:], EYE_d[:])

            sums_sb = sumspool.tile([128, NGROUPS], f32)

            # persistent precise-chunk tiles (3 rotating buffers):
            #   rows 0:32  = x^2     (GpSimd, per tile)
            #   rows 32:64 = x       (DMA'd per tile)
            #   row 64     = ones    (PAD, written once)
            #   rows 65:128= zeros   (PAD, written once)
            NC4 = 3
            c4_tiles = []
            for i in range(NC4):
                c4 = xt4pool.tile(
                    [128, TILE_ROWS], f32r, tag=f"c4P{i}", bufs=1, name=f"c4_p{i}"
                )
                nc.scalar.dma_start(c4[64:128, :], PAD_d[:])
                c4_tiles.append(c4)

            # 3-stage software pipeline:
            #   A(t):   input DMAs + GpSimd x^2 products for tile t
            #   B(t-1): selection matmuls (PE) + fp8 pair products (DVE)
            #   C(t-2): main accumulating matmuls (PE) + exp (ACT)
            stageA = {}
            stageB = {}
            for t in range(NTILES + 2):
                if t < NTILES:
                    cols = slice(t * TILE_ROWS, (t + 1) * TILE_ROWS)
                    xt4_t = xt4pool.tile([128, TILE_ROWS], f32r, tag="xt4")
                    nc.sync.dma_start(xt4_t[:], X4_d[:, cols])
                    c4_t = c4_tiles[t % NC4]
                    nc.sync.dma_start(c4_t[32:64, :], X4_d[0:32, cols])
                    # diagonal x^2 products on the idle GpSimd/Pool queue
                    nc.gpsimd.tensor_mul(
                        c4_t[0:32, :], xt4_t[0:32, :], xt4_t[0:32, :]
                    )
                    stageA[t] = (xt4_t, c4_t)

                tb = t - 1
                if 0 <= tb < NTILES:
                    xt4_b, c4_b = stageA.pop(tb)
                    # rotated copies via K=128 selection matmuls; DVE builds
                    # fp8 DoubleRow product bundles [128, 2, rows]
                    prod8 = []
                    for c in range(2):
                        p8 = xxpool.tile([128, 2, TILE_ROWS], f8, tag=f"p8{c}")
                        prod8.append(p8)
                    xt4_bc = xt4_b[:].unsqueeze(1).broadcast_to((128, 2, TILE_ROWS))
                    for c in range(2):
                        rotps = ps_rot.tile([128, 2, TILE_ROWS], f32, tag="rot")
                        for k in range(2):
                            g = 2 * c + k
                            nc.tensor.matmul(
                                rotps[:, k, :],
                                SEL_sb[:, 128 * g : 128 * (g + 1)],
                                xt4_b[:],
                                start=True,
                                stop=True,
                            )
                        nc.vector.tensor_mul(prod8[c][:], xt4_bc, rotps[:])
                    stageB[tb] = (prod8, c4_b)

                tcm = t - 2
                if tcm >= 0:
                    prod8, c4_b = stageB.pop(tcm)
                    # main matmuls for tile t-2: all f32r first, then all fp8
                    # (PE dtype-mode switches serialize the pipeline)
                    pstiles = []
                    for half in range(2):
                        psmain = ps_main.tile([128, 2 * M], f32, tag="main")
                        pstiles.append(psmain)
                        for s2 in range(2):
                            sub = half * 2 + s2
                            # start=True arms zero-on-first-touch for the WHOLE
                            # 2KB bank; arm it once (s2=0) - s2=1's first write
                            # consumes the pending-zero of its own bytes.
                            # K=65: rows 65:128 of the precise chunk are all
                            # zero coefficients - a shorter stationary load
                            nc.tensor.matmul(
                                psmain[:, s2 * M : (s2 + 1) * M],
                                c4_b[0:65, sub * 128 : (sub + 1) * 128],
                                UP_sb[0:65, :],
                                start=(s2 == 0),
                                stop=False,
                                skip_group_check=True,
                            )
                    for half in range(2):
                        psmain = pstiles[half]
                        for s2 in range(2):
                            sub = half * 2 + s2
                            for c in range(2):
                                nc.tensor.matmul(
                                    psmain[:, s2 * M : (s2 + 1) * M],
                                    prod8[c][:, :, sub * 128 : (sub + 1) * 128],
                                    U8_sb[:, c],
                                    perf_mode=DR,
                                    start=False,
                                    stop=(c == 1),
                                )
                    for half in range(2):
                        psmain = pstiles[half]
                        for s2 in range(2):
                            sub = half * 2 + s2
                            expsc = exppool.tile([128, M], f32, tag="exp")
                            col = tcm * 4 + sub
                            nc.scalar.activation(
                                expsc[:],
                                psmain[:, s2 * M : (s2 + 1) * M],
                                AF.Exp,
                                accum_out=sums_sb[:, col : col + 1],
                            )

            # epilogue: ll^T = Ln(sums); transpose; contiguous DMA out
            llT = finpool.tile([128, NGROUPS], f32)
            nc.scalar.activation(llT[:], sums_sb[:], AF.Ln)
            llps = ps_rot.tile([128, 128], f32, tag="rot")
            nc.tensor.transpose(llps[:], llT[:], EYE_sb[:])
            ll_sb = finpool.tile([128, 128], f32)
            nc.scalar.copy(ll_sb[:], llps[:])
            nc.sync.dma_start(OUT_d.rearrange("(c p) -> c p", c=128), ll_sb[:])

    nc.compile()
    return nc


def _host_prep(center, cov_inv_sqrt, weight, threshold):
    import ml_dtypes

    L = np.asarray(cov_inv_sqrt, dtype=np.float64)
    w = np.abs(np.asarray(weight, dtype=np.float64))
    pr = w / w.sum()
    A = np.einsum("mij,mkj->mik", L, L)
    sign, logdet = np.linalg.slogdet(A)
    logcoef = np.log(pr) + 0.5 * logdet
    c64 = np.asarray(center, dtype=np.float64)
    Ac = np.einsum("mkl,ml->mk", A, c64)
    term3 = np.einsum("mk,mk->m", c64, Ac)
    bias = logcoef - 0.5 * term3 - float(np.asarray(threshold).reshape(-1)[0])

    d = np.arange(32)
    f8 = ml_dtypes.float8_e4m3

    # precise chunk: diag x^2 rows, x-linear rows, bias row
    UP = np.zeros((128, M), np.float32)
    UP[0:32, :] = (-0.5 * A[:, d, d].T).astype(np.float32)
    UP[32:64, :] = Ac.T.astype(np.float32)
    UP[64, :] = bias.astype(np.float32)

    # fp8 bundles: shift groups g = 2c + k cover shifts 4g+1 .. 4g+4;
    # partition blocks 0..2 use plain-x left factors (shifts 4g+1..4g+3),
    # block 3 uses the rot16 left factor (shift 4g+4).
    U8 = np.zeros((128, 2, 2, M), np.float32)
    SEL = np.zeros((128, 512), np.float32)
    for g in range(4):
        c, k = divmod(g, 2)
        for blk in range(4):
            if blk < 3:
                s = 4 * g + blk + 1
                a = d
                b = (d + s) % 32
            else:
                s = 4 * g + 4
                a = (d + 16) % 32
                b = (a + s) % 32
            mult = 1.0 if s == 16 else 2.0
            U8[32 * blk + d, c, k, :] = (-0.5 * mult * A[:, a, b].T).astype(
                np.float32
            )
            SEL[b, 128 * g + 32 * blk + d] = 1.0
    U8 = U8.astype(f8)

    PAD = np.zeros((64, TILE_ROWS), np.float32)
    PAD[0, :] = 1.0
    EYE = np.eye(128, dtype=np.float32)
    return UP, U8, SEL, PAD, EYE


def _host_x4t(X):
    """[128, N]: rows 0:96 = three copies of X^T, rows 96:128 = rot16(X^T)."""
    X4T = np.empty((128, X.shape[0]), np.float32)
    XT = X.T
    X4T[0:32] = XT
    X4T[32:64] = XT
    X4T[64:96] = XT
    X4T[96:128] = XT[(np.arange(32) + 16) % 32]
    return X4T


def kernel(X, center, cov_inv_sqrt, weight, threshold):
    global _PROGRAM
    from concourse.bass_utils import run_bass_kernel_spmd

    X = np.ascontiguousarray(np.asarray(X, dtype=np.float32))
    UP, U8, SEL, PAD, EYE = _host_prep(center, cov_inv_sqrt, weight, threshold)
    X4T = _host_x4t(X)

    if _PROGRAM is None:
        _PROGRAM = _build_program()
    nc = _PROGRAM

    in_maps = []
    for k in range(NCORES):
        in_maps.append(
            {
                "X4": np.ascontiguousarray(
                    X4T[:, k * NC_ROWS : (k + 1) * NC_ROWS]
                ),
                "UP": UP,
                "U8": U8,
                "SEL": SEL,
                "PAD": PAD,
                "EYE": EYE,
            }
        )
    res = run_bass_kernel_spmd(nc, in_maps, list(range(NCORES)))
    out = np.concatenate([res.results[k]["out"] for k in range(NCORES)])
    return out.astype(np.float32)
